# revision 2
# baseline (speedup 1.0000x reference)
"""Trainium2 Bass kernel for nn_LinearFlowModel (dense_mlp), v4.

Same algorithm as v2 (host-transposed fp16 stateT streamed against stationary
W halves; fp16 in/out; ScalarE/VectorE evacuate PSUM with the bias add), with
the pipeline restructured from the v3 trace: an out-DMA's descriptor-gen
blocks its issuing sequencer until the evacuation writing that tile has
completed, so DGE must not share a sequencer with evacuation compute.

  - ALL out-DMAs (both halves) issue from the Sync sequencer, whose waits
    idle no compute engine. The Scalar sequencer stream is input DGEs (no
    waits -- every input chunk has its own buffer) followed by pure ACT
    evacuation ops, so the ACT engine is never starved by a DMA wait.
  - Input chunks alternate between the two rings and are all issued up
    front; chunk 0 goes first (critical path), w2/bias after it.
  - delta and var are separate SBUF tiles and separate DRAM tensors, so each
    half's out-DMA only waits on its own evacuation engine.
"""

import os
import sys

if "/opt/trn_rl_repo" not in sys.path:
    sys.path.insert(0, "/opt/trn_rl_repo")

import numpy as np

B = 131072
D = 128
NCORES = 8
BLOC = B // NCORES  # 16384 batch columns per core

IN_BLK = int(os.environ.get("KV4_IN_BLK", "2048"))  # input DMA chunk (0.5 MB)
CB = int(os.environ.get("KV4_CB", "1024"))  # compute block: 2 PSUM banks/half
OUT_BLK = int(os.environ.get("KV4_OUT_BLK", "2048"))  # per-half out DMA chunk
YBUFS = int(os.environ.get("KV4_YBUFS", "5"))
PSBUFS = int(os.environ.get("KV4_PSBUFS", "4"))

assert BLOC % IN_BLK == 0 and BLOC % OUT_BLK == 0 and OUT_BLK % CB == 0
assert CB % 512 == 0

_prog = None


def _build_program():
    import concourse.bacc as bacc
    import concourse.mybir as mybir
    from concourse import tile

    f32 = mybir.dt.float32
    f16 = mybir.dt.float16

    nc = bacc.Bacc(
        "TRN2",
        target_bir_lowering=False,
        debug=False,
        num_devices=NCORES,
    )

    xT_d = nc.dram_tensor("xT", [D, BLOC], f16, kind="ExternalInput").ap()
    w2_d = nc.dram_tensor("w2", [D, 2, D], f16, kind="ExternalInput").ap()
    bias_d = nc.dram_tensor("bias", [D, 2], f32, kind="ExternalInput").ap()
    dT_d = nc.dram_tensor("dT", [D, BLOC], f16, kind="ExternalOutput").ap()
    vT_d = nc.dram_tensor("vT", [D, BLOC], f16, kind="ExternalOutput").ap()

    n_in = BLOC // IN_BLK

    with tile.TileContext(nc) as tc:
        with (
            tc.tile_pool(name="const", bufs=1) as cpool,
            tc.tile_pool(name="xin", bufs=n_in) as xpool,
            tc.tile_pool(name="yd", bufs=YBUFS) as ydpool,
            tc.tile_pool(name="yv", bufs=YBUFS) as yvpool,
            tc.tile_pool(name="ps", bufs=PSBUFS, space="PSUM") as pspool,
        ):
            # x chunk 0 first (critical path), then the tiny consts, then the
            # rest of the input stream, alternating rings so DGE parallelizes.
            # All input DGEs precede any out-DMA (sync) / ACT op (scalar) in
            # their sequencer streams and have no waits.
            xts = []
            x0 = xpool.tile([D, IN_BLK], f16, tag="x")
            nc.sync.dma_start(x0[:], xT_d[:, 0:IN_BLK])
            xts.append(x0)
            w_sb = cpool.tile([D, 2, D], f16)
            nc.scalar.dma_start(w_sb[:], w2_d[:])
            bias_sb = cpool.tile([D, 2], f32)
            nc.scalar.dma_start(bias_sb[:], bias_d[:])
            for ib in range(1, n_in):
                x = xpool.tile([D, IN_BLK], f16, tag="x")
                eng = nc.scalar if ib % 2 else nc.sync
                eng.dma_start(x[:], xT_d[:, ib * IN_BLK : (ib + 1) * IN_BLK])
                xts.append(x)

            for ob in range(BLOC // OUT_BLK):
                yd = ydpool.tile([D, OUT_BLK], f16, tag="yd")
                yv = yvpool.tile([D, OUT_BLK], f16, tag="yv")
                for cb in range(OUT_BLK // CB):
                    boff = ob * OUT_BLK + cb * CB  # global batch-col offset
                    yc = cb * CB
                    for o in range(2):
                        ps = pspool.tile([D, CB], f32)
                        for k in range(CB // 512):
                            goff = boff + k * 512
                            x = xts[goff // IN_BLK]
                            xo = goff % IN_BLK
                            nc.tensor.matmul(
                                ps[:, k * 512 : (k + 1) * 512],
                                w_sb[:, o, :],
                                x[:, xo : xo + 512],
                                start=True,
                                stop=True,
                            )
                        if o == 0:
                            nc.scalar.add(yd[:, yc : yc + CB], ps[:], bias_sb[:, 0:1])
                        else:
                            nc.vector.tensor_scalar_add(
                                yv[:, yc : yc + CB], ps[:], bias_sb[:, 1:2]
                            )
                off = ob * OUT_BLK
                nc.sync.dma_start(dT_d[:, off : off + OUT_BLK], yd[:])
                nc.sync.dma_start(vT_d[:, off : off + OUT_BLK], yv[:])

    nc.compile()
    return nc


def _get_program():
    global _prog
    if _prog is None:
        _prog = _build_program()
    return _prog


def _prep_inputs(state, W, b):
    state = np.asarray(state, dtype=np.float32)
    W = np.asarray(W, dtype=np.float32)
    b = np.asarray(b, dtype=np.float32)
    w2 = np.ascontiguousarray(W.transpose(2, 1, 0), dtype=np.float16)  # [d, o, n]
    bias = np.ascontiguousarray(b)  # [n, o] -- partition = node = out row m
    state16 = state.astype(np.float16)
    in_maps = []
    for i in range(NCORES):
        xT = np.ascontiguousarray(state16[i * BLOC : (i + 1) * BLOC, :].T)
        in_maps.append({"xT": xT, "w2": w2, "bias": bias})
    return in_maps


def run_on_device(state, W, b, trace=False, **kw):
    """Run the Bass kernel on the 8 NeuronCores; returns (delta, var, results)."""
    from concourse.bass_utils import run_bass_kernel_spmd

    nc = _get_program()
    in_maps = _prep_inputs(state, W, b)
    res = run_bass_kernel_spmd(nc, in_maps, list(range(NCORES)), trace=trace, **kw)
    delta = np.empty((B, D), dtype=np.float32)
    var = np.empty((B, D), dtype=np.float32)
    for i, r in enumerate(res.results):
        delta[i * BLOC : (i + 1) * BLOC] = r["dT"].T
        var[i * BLOC : (i + 1) * BLOC] = r["vT"].T
    return delta, var, res


def kernel(state, W, b):
    try:
        delta, var, _ = run_on_device(state, W, b, trace=False)
    except Exception:
        delta, var, _ = run_on_device(state, W, b, trace=False)
    return delta, var
